# revision 47
# baseline (speedup 1.0000x reference)
"""Trainium2 Bass kernel for nn_BlockAttnRes (block-softmax residual net).

Shapes: embedding [8, 8192, 128] f32, L=16 layers, BLOCK_SIZE=4.
Sharding: batch dim B=8 across 8 cores (1 batch row / core = 8192 tokens).

Per-core: tokens-on-partitions bf16 state resident in SBUF.
6 state slots: slot0 = emb, slot 1+g = partial of group g (becomes block g+1
at commit). Loop: For_i over token tiles (F=512 tokens = 4 blocks of 128),
python-unrolled 16 layers inside. Key techniques:
  - softmax denominator never materialized: LayerNorm is scale-invariant, so
    the unnormalized h~ = sum_i E_i V_i feeds LN directly (den only enters an
    eps correction and the final layer's output scaling).
  - exp via tanh identity e^t=(1+T)/(1-T) (gelu ACT table set only)
  - running partial's logits vs EVERY layer's w via one PE matmul per layer
    on the transposed MLP output, accumulated across layers directly in PSUM
    (start=False accumulation; all 4 streams stacked in one PSUM bank at
    partition offsets 0/32/64/96). At block-commit the accumulated dots ARE
    the new block's static logits, so creation needs no transposes/matmuls.
  - rsqrt via int bit-trick seed + Newton iterations (DVE only)
  - weighted sums: per-block fused stt chains, last link's accum_out gives
    hsum free; one block's chain runs on the idle Pool engine.
  - LayerNorm affine folded into W1' = diag(g)@W1, b1' = b1 + ln_b@W1 (host)
  - MLP: PE transposes to col layout, bf16 matmuls, ACT gelu fused bias
  - partial accumulated in PSUM by f32r transpose-matmuls (start=False accum)
"""
import contextlib
import ctypes
import sys
import types
from contextlib import ExitStack

sys.path.insert(0, "/opt/trn_rl_repo")


def _install_ntff_hook():
    """Provide antenv.axon_hooks (missing in the trimmed repo) so
    run_bass_kernel_spmd(trace=True) can collect NTFF profiles."""
    if "antenv.axon_hooks" in sys.modules:
        return
    try:
        lib = ctypes.CDLL("/opt/axon/libaxon_pjrt.so")
    except OSError:
        return
    if not hasattr(lib, "axon_start_nrt_profile"):
        hook = None
    else:
        lib.axon_start_nrt_profile.argtypes = [
            ctypes.POINTER(ctypes.c_int64), ctypes.c_size_t]
        lib.axon_start_nrt_profile.restype = ctypes.c_int64
        lib.axon_stop_nrt_profile.argtypes = [ctypes.c_char_p]
        lib.axon_stop_nrt_profile.restype = ctypes.c_int64

        @contextlib.contextmanager
        def hook(output_dir, device_ids):
            import jax
            jax.devices()
            if device_ids:
                ids = (ctypes.c_int64 * len(device_ids))(*device_ids)
                rc = lib.axon_start_nrt_profile(ids, len(device_ids))
            else:
                rc = lib.axon_start_nrt_profile(None, 0)
            if rc != 0:
                raise RuntimeError(f"axon_start_nrt_profile rc={rc}")
            try:
                yield
            finally:
                n = lib.axon_stop_nrt_profile(str(output_dir).encode())
                print(f"profile: {n} file(s) -> {output_dir}", file=sys.stderr)

    mod = types.ModuleType("antenv.axon_hooks")
    mod.get_axon_ntff_profile_hook = lambda: hook
    mod.set_axon_ntff_profile_hook = lambda h: None
    sys.modules["antenv.axon_hooks"] = mod

import numpy as np
import ml_dtypes

import concourse.bacc as bacc
import concourse.bass as bass
import concourse.mybir as mybir
from concourse.bass_utils import run_bass_kernel_spmd
from concourse.tile import TileContext
from concourse.masks import make_identity

F32 = mybir.dt.float32
BF16 = mybir.dt.bfloat16
F32R = mybir.dt.float32r
I32 = mybir.dt.int32
ALU = mybir.AluOpType
AF = mybir.ActivationFunctionType
AX = mybir.AxisListType

L = 16
GROUP = 4
D = 128
NBLK = 4                 # 128-token blocks per tile
F = NBLK * 128           # tokens per tile
EPS_RMS = 1e-8
EPS_LN = 1e-5
MAGIC = 0x5F3759DF
N_CORES = 8

# --- tunable engine placement ------------------------------------------
POOL_WSUM_BLKS = 1       # how many of the 4 blocks' wsum chains go to Pool
HSSQ_SQ = "act"          # square for LN variance: "act" | "dve" | "pool"
SSQP_SQ = "pool"          # square for partial rms
VTT_SPLIT = True         # alternate vtT copy between ACT and DVE

_CACHE = {}


def _bcast(ap, n):
    """Append a stride-0 inner free dim of size n to an AP."""
    return bass.AP(tensor=ap.tensor, offset=ap.offset,
                   ap=list(ap.ap) + [[0, n]])


def _newton_rsqrt(nc, pool, x, shape, iters=2):
    """y = rsqrt(x) for x [128, *shape] f32 tile (positive). DVE-only:
    int bit-trick seed + Newton iterations. (ACT Sqrt would force a
    1.3us activation-table reload against the resident gelu table.)"""
    y = pool.tile([128] + list(shape), F32, tag="nw_y", name="nw_y")
    xi = x.bitcast(I32)
    yi = y.bitcast(I32)
    nc.vector.tensor_scalar(out=yi[:], in0=xi[:], scalar1=1, scalar2=0,
                            op0=ALU.logical_shift_right,
                            op1=ALU.logical_shift_right)
    nc.vector.tensor_scalar(out=yi[:], in0=yi[:], scalar1=-1, scalar2=MAGIC,
                            op0=ALU.mult, op1=ALU.add)
    t = pool.tile([128] + list(shape), F32, tag="nw_t", name="nw_t")
    for _ in range(iters):
        nc.vector.tensor_mul(t[:], y[:], y[:])
        nc.vector.scalar_tensor_tensor(out=t[:], in0=t[:], scalar=-0.5,
                                       in1=x[:], op0=ALU.mult, op1=ALU.mult)
        nc.vector.scalar_tensor_tensor(out=y[:], in0=t[:], scalar=1.5,
                                       in1=y[:], op0=ALU.add, op1=ALU.mult)
    return y


def build(tiles_per_core=16):
    nc = bacc.Bacc("TRN2", target_bir_lowering=False)
    n_tok = tiles_per_core * F

    embh = nc.dram_tensor("embh", [n_tok, D], BF16, kind="ExternalInput")
    w_tf = nc.dram_tensor("w_tf", [D, L], F32, kind="ExternalInput")
    w_tb = nc.dram_tensor("w_tb", [D, L], BF16, kind="ExternalInput")
    w1p = nc.dram_tensor("w1p", [D, L * 2 * 128], BF16, kind="ExternalInput")
    b1p = nc.dram_tensor("b1p", [128, 2 * L], F32, kind="ExternalInput")
    w2p = nc.dram_tensor("w2p", [128, L * 2 * D], BF16, kind="ExternalInput")
    out = nc.dram_tensor("out", [n_tok, D], F32, kind="ExternalOutput")

    emb_v = embh.rearrange("(T b p) d -> T p b d", b=NBLK, p=128)
    out_v = out.rearrange("(T b p) d -> T p b d", b=NBLK, p=128)

    with TileContext(nc) as tc, ExitStack() as es:
        cst = es.enter_context(tc.tile_pool(name="cst", bufs=1))
        ident16 = cst.tile([128, 128], BF16)
        make_identity(nc, ident16[:])
        identf = cst.tile([128, 128], F32)
        make_identity(nc, identf[:])
        identr = cst.tile([128, 128], F32R)
        nc.vector.tensor_copy(identr[:], identf[:])

        wT_f = cst.tile([128, L], F32)
        nc.sync.dma_start(wT_f[:], w_tf[:])
        wT_b = cst.tile([128, L], BF16)
        nc.sync.dma_start(wT_b[:], w_tb[:])
        # per-stream banded [128, 64] lhsT: w.T in cols 16k..16k+15, 0 else
        wT64_f = cst.tile([128, 4, 64], F32)
        nc.vector.memset(wT64_f[:], 0.0)
        for k4 in range(4):
            nc.vector.tensor_copy(wT64_f[:, k4, 16 * k4:16 * k4 + L], wT_f[:])
        wT64 = cst.tile([128, 4, 64], F32R)
        nc.vector.tensor_copy(wT64[:], wT64_f[:])

        w1p_sb = cst.tile([128, L, 2, 128], BF16)
        nc.sync.dma_start(w1p_sb[:], w1p[:].rearrange(
            "d (l h m) -> d l h m", l=L, h=2))
        b1p_sb = cst.tile([128, 2 * L], F32)
        nc.sync.dma_start(b1p_sb[:], b1p[:])
        w2p_sb = cst.tile([128, L, 2, D], BF16)
        nc.sync.dma_start(w2p_sb[:], w2p[:].rearrange(
            "m (l k d) -> m l k d", l=L, k=2))

        sp = es.enter_context(tc.tile_pool(name="state", bufs=4))
        big = es.enter_context(tc.tile_pool(name="big", bufs=5))
        sml = es.enter_context(tc.tile_pool(name="sml", bufs=12))
        nwp = es.enter_context(tc.tile_pool(name="nw", bufs=12))
        pp_big = es.enter_context(tc.tile_pool(name="pp_big", bufs=3, space="PSUM"))
        pp_par = es.enter_context(tc.tile_pool(name="pp_par", bufs=4, space="PSUM"))
        pp_dots = es.enter_context(tc.tile_pool(name="pp_dots", bufs=1, space="PSUM"))

        NS = 4 if tiles_per_core % 4 == 0 else (
            2 if tiles_per_core % 2 == 0 else 1)

        def sq_engine(which):
            return {"act": None, "dve": nc.vector, "pool": nc.gpsimd}[which]

        def emit_square(which, out_ap, in_ap):
            if which == "act":
                nc.scalar.activation(out_ap, in_ap, AF.Square)
            else:
                eng = nc.vector if which == "dve" else nc.gpsimd
                eng.tensor_tensor(out_ap, in_ap, in_ap, op=ALU.mult)

        def tile_start(it, k, sh):
            st = {"it": it, "k": k, "sh": sh}
            st["slots"] = sp.tile([128, 6, NBLK, D], BF16, tag="slots",
                                  name="slots")
            nc.sync.dma_start(st["slots"][:, 0, :, :], emb_v[bass.ds(it, 1)])
            st["partial_ps"] = pp_par.tile([128, NBLK, D], F32R, tag="par",
                                           name="par")
            return st

        def stats_finish(sts, s_idx, sh, stats_cr, ms_cr):
            """Batched: rms from ms + scaled dots -> sdots_all[s_idx]."""
            ns_ = len(sts)
            xs = sml.tile([128, NS, NBLK], F32, tag="xs_cr")
            nc.vector.tensor_scalar(out=xs[:, 0:ns_], in0=ms_cr[:, 0:ns_],
                                    scalar1=1.0 / D, scalar2=EPS_RMS,
                                    op0=ALU.mult, op1=ALU.add)
            rms = _newton_rsqrt(nc, nwp, xs, (NS, NBLK), iters=1)
            nc.vector.scalar_tensor_tensor(
                out=sh["sdots_all"][:, 0:ns_, s_idx, :, :],
                in0=stats_cr[:, 0:ns_, :, 0:L],
                scalar=1.0, in1=_bcast(rms[:, 0:ns_], L),
                op0=ALU.bypass, op1=ALU.mult)

        def emb_creation(sts, sh):
            """Stats for slot 0 (embedding): transpose + dots matmul +
            token-layout mean-square."""
            stats_cr = sml.tile([128, NS, NBLK, L], F32, tag="stats_cr")
            ms_cr = sml.tile([128, NS, NBLK], F32, tag="ms_cr")
            for st in sts:
                k = st["k"]
                srcT_ps = pp_big.tile([128, F], BF16, tag="big_ps",
                                      name="srcT_ps")
                for blk in range(NBLK):
                    nc.tensor.matmul(srcT_ps[:, blk * 128:(blk + 1) * 128],
                                     st["slots"][:, 0, blk, :], ident16[:],
                                     is_transpose=True, start=True, stop=True,
                                     skip_group_check=True)
                srcT = big.tile([128, F], BF16, tag="srcT", name="srcT")
                nc.scalar.copy(srcT[:], srcT_ps[:])
                dots_ps = pp_big.tile([L, F], F32, tag="big_ps",
                                      name="dots_ps")
                nc.tensor.matmul(dots_ps[:], wT_b[:], srcT[:],
                                 start=True, stop=True, skip_group_check=True)
                dots_sb = big.tile([L, F], F32, tag="dots_sb",
                                   name="dots_sb")
                nc.scalar.copy(dots_sb[:], dots_ps[:])
                statT_ps = pp_big.tile([128, NBLK, L], F32, tag="big_ps",
                                       name="statT_ps")
                for c in range(NBLK):
                    nc.tensor.matmul(statT_ps[:, c, :],
                                     dots_sb[:, c * 128:(c + 1) * 128],
                                     identf[0:L, 0:L],
                                     is_transpose=True, start=True, stop=True,
                                     skip_group_check=True)
                nc.vector.tensor_copy(stats_cr[:, k], statT_ps[:])
                sqe = big.tile([128, NBLK, D], BF16, tag="sq", name="sqe", bufs=8)
                emit_square("act", sqe[:], st["slots"][:, 0, :, :])
                nc.vector.tensor_reduce(ms_cr[:, k], sqe[:],
                                        axis=AX.X, op=ALU.add)
            stats_finish(sts, 0, sh, stats_cr, ms_cr)

        def commit_creation(sts, s_idx, sh, dots_psum):
            """Block commit: the accumulated dots ARE the new block's static
            logits; only rms (token-layout) + transpose needed."""
            ns_ = len(sts)
            stats_cr = sml.tile([128, NS, NBLK, L], F32, tag="stats_cr")
            ms_cr = sml.tile([128, NS, NBLK], F32, tag="ms_cr")
            drow = big.tile([64, F], F32, tag="drow", name="drow")
            nc.scalar.copy(drow[:], dots_psum[:])
            statT_ps = pp_big.tile([128, NBLK, 64], F32, tag="big_ps",
                                   name="statT_ps")
            for c in range(NBLK):
                nc.tensor.matmul(statT_ps[:, c, :],
                                 drow[:, c * 128:(c + 1) * 128],
                                 identf[0:64, 0:64],
                                 is_transpose=True, start=True, stop=True,
                                 skip_group_check=True)
            sp_ap = statT_ps[:]
            nc.vector.tensor_copy(
                stats_cr[:, 0:ns_],
                bass.AP(tensor=sp_ap.tensor, offset=sp_ap.offset,
                        ap=[sp_ap.ap[0], [16, ns_], [64, NBLK], [1, L]]))
            for st in sts:
                sqc = big.tile([128, NBLK, D], BF16, tag="sq", name="sqc", bufs=8)
                nc.scalar.activation(sqc[:], st["slots"][:, s_idx, :, :],
                                     AF.Square)
                nc.vector.tensor_reduce(ms_cr[:, st["k"]], sqc[:],
                                        axis=AX.X, op=ALU.add)
            stats_finish(sts, s_idx, sh, stats_cr, ms_cr)

        def _exp_piece(E_T, Bt, R, E, lo, hi, ns_):
            """E[.., lo:hi] = exp from T = tanh(logit/2)."""
            nc.vector.tensor_scalar(out=Bt[:, 0:ns_, :, lo:hi],
                                    in0=E_T[:, 0:ns_, :, lo:hi],
                                    scalar1=-1.0, scalar2=-1.0,
                                    op0=ALU.mult, op1=ALU.subtract)
            nc.vector.reciprocal(R[:, 0:ns_, :, lo:hi],
                                 Bt[:, 0:ns_, :, lo:hi])
            nc.vector.tensor_scalar(out=E[:, 0:ns_, :, lo:hi],
                                    in0=R[:, 0:ns_, :, lo:hi],
                                    scalar1=2.0, scalar2=-1.0,
                                    op0=ALU.mult, op1=ALU.add)

        def layer_front(sts, l, sh):
            """Static part of a layer: softmax statics + static chain links.
            Depends only on committed slots/sdots, so it can be emitted ahead
            to keep engines fed while the previous layer's cross-engine
            round-trips resolve."""
            ns_ = len(sts)
            g, j = l // GROUP, l % GROUP
            nsrc = g + 1
            has_p = j > 0
            n = nsrc + (1 if has_p else 0)
            last = l == L - 1
            sdots_all = sh["sdots_all"]

            # at group-start layers the newest block's stats come from the
            # commit path of the previous layer; defer that source (like a
            # partial link) so the other chains don't wait on the commit.
            late_static = False  # deferring the new block measured slower
            n_early = nsrc - 1 if late_static else nsrc
            E_T = sml.tile([128, NS, NBLK, 5], F32, tag="E_T")
            e_ap = E_T[:]
            statics_out = bass.AP(
                tensor=e_ap.tensor, offset=e_ap.offset,
                ap=[e_ap.ap[0], [NBLK * 5, ns_], [1, n_early], [5, NBLK]])
            nc.scalar.activation(out=statics_out,
                                 in_=sdots_all[:, 0:ns_, 0:n_early, :, l],
                                 func=AF.Tanh, scale=0.5)
            Bt = sml.tile([128, NS, NBLK, 5], F32, tag="B")
            R = sml.tile([128, NS, NBLK, 5], F32, tag="R")
            E = sml.tile([128, NS, NBLK, 5], F32, tag="E")
            _exp_piece(E_T, Bt, R, E, 0, n_early, ns_)

            fr = {"E_T": E_T, "Bt": Bt, "R": R, "E": E, "n": n,
                  "nsrc": nsrc, "has_p": has_p, "last": last,
                  "late_static": late_static}
            if last:
                return fr

            hsum = sml.tile([128, NS, NBLK], F32, tag="hsum")
            pool_blks = (range(NBLK - POOL_WSUM_BLKS, NBLK)
                         if n >= 3 else ())
            hs = []
            for st in sts:
                hs.append(big.tile([128, NBLK, D], BF16, tag="h",
                                   name="h", bufs=8))
            fr["hsum"], fr["hs"], fr["pool_blks"] = hsum, hs, pool_blks
            # phase 1: first links
            for st, h in zip(sts, hs):
                k, slots = st["k"], st["slots"]
                for blk in range(NBLK):
                    if n == 1:
                        nc.vector.tensor_scalar(
                            out=h[:, blk, :], in0=slots[:, 0, blk, :],
                            scalar1=E[:, k, blk, 0:1], scalar2=0.0,
                            op0=ALU.mult, op1=ALU.add,
                            accum_out=hsum[:, k, blk:blk + 1])
                    elif blk in pool_blks:
                        nc.gpsimd.tensor_tensor(
                            h[:, blk, :], slots[:, 0, blk, :],
                            _bcast(E[:, k, blk, 0], 128), op=ALU.mult)
                    else:
                        nc.vector.tensor_scalar(
                            out=h[:, blk, :], in0=slots[:, 0, blk, :],
                            scalar1=E[:, k, blk, 0:1], scalar2=None,
                            op0=ALU.mult)
            # phase 2: static mid links (i in 1..n-2 are always static)
            for st, h in zip(sts, hs):
                k, slots = st["k"], st["slots"]
                for blk in range(NBLK):
                    if blk in pool_blks:
                        for i in range(1, n - 1):
                            wt = big.tile([128, 128], BF16, tag="wtmp",
                                          name="wtmp")
                            nc.gpsimd.tensor_tensor(
                                wt[:], slots[:, i, blk, :],
                                _bcast(E[:, k, blk, i], 128), op=ALU.mult)
                            nc.gpsimd.tensor_tensor(
                                h[:, blk, :], h[:, blk, :], wt[:],
                                op=ALU.add)
                    else:
                        for i in range(1, n - 1):
                            nc.vector.scalar_tensor_tensor(
                                out=h[:, blk, :], in0=slots[:, i, blk, :],
                                scalar=E[:, k, blk, i:i + 1],
                                in1=h[:, blk, :],
                                op0=ALU.mult, op1=ALU.add)
            # static last link: only when nothing is deferred
            if n > 1 and not has_p and not late_static:
                for st, h in zip(sts, hs):
                    k, slots = st["k"], st["slots"]
                    for blk in range(NBLK):
                        nc.vector.scalar_tensor_tensor(
                            out=h[:, blk, :], in0=slots[:, n - 1, blk, :],
                            scalar=E[:, k, blk, n - 1:n],
                            in1=h[:, blk, :],
                            op0=ALU.mult, op1=ALU.add,
                            accum_out=hsum[:, k, blk:blk + 1])
            return fr

        def layer_back(sts, l, sh, dots_psum, fr, filler=None):
            ns_ = len(sts)
            g, j = l // GROUP, l % GROUP
            nsrc = g + 1
            has_p = j > 0
            n = nsrc + (1 if has_p else 0)
            last = l == L - 1
            E_T, E = fr["E_T"], fr["E"]

            # --- partial-source logit from PSUM-accumulated dots ---
            # producers on ACT/PE/Pool first, then the next layer's static
            # work as filler, then the DVE consumers (so the in-order DVE
            # queue has useful work while squares/copies land).
            if has_p:
                dotp = sml.tile([128, NS, NBLK], F32, tag="dotp")
                ssqp = sml.tile([128, NS, NBLK], F32, tag="ssqp")
                drow = big.tile([64, F], F32, tag="drow", name="drow")
                nc.scalar.copy(drow[:], dots_psum[:])
                statp_ps = pp_big.tile([128, NBLK, 64], F32, tag="big_ps",
                                       name="statp_ps")
                for c in range(NBLK):
                    nc.tensor.matmul(statp_ps[:, c, :],
                                     drow[:, c * 128:(c + 1) * 128],
                                     identf[0:64, 0:64],
                                     is_transpose=True, start=True,
                                     stop=True, skip_group_check=True)
                sqps = []
                for st in sts:
                    sqp = big.tile([128, NBLK, D], BF16, tag="sq", name="sqp", bufs=8)
                    emit_square(SSQP_SQ, sqp[:], st["slots"][:, nsrc, :, :])
                    sqps.append(sqp)
            nxt = filler() if filler is not None else None
            if has_p:
                sp_ap = statp_ps[:]
                nc.vector.tensor_copy(
                    dotp[:, 0:ns_],
                    bass.AP(tensor=sp_ap.tensor, offset=sp_ap.offset + l,
                            ap=[sp_ap.ap[0], [16, ns_], [64, NBLK]]))
                for st, sqp in zip(sts, sqps):
                    nc.vector.tensor_reduce(ssqp[:, st["k"]], sqp[:],
                                            axis=AX.X, op=ALU.add)
                xp = sml.tile([128, NS, NBLK], F32, tag="xp")
                nc.vector.tensor_scalar(out=xp[:, 0:ns_], in0=ssqp[:, 0:ns_],
                                        scalar1=1.0 / D, scalar2=EPS_RMS,
                                        op0=ALU.mult, op1=ALU.add)
                rmsp = _newton_rsqrt(nc, nwp, xp, (NS, NBLK), iters=1)
                lp = sml.tile([128, NS, NBLK], F32, tag="lp")
                nc.vector.tensor_mul(lp[:, 0:ns_], dotp[:, 0:ns_],
                                     rmsp[:, 0:ns_])
                nc.scalar.activation(out=E_T[:, 0:ns_, :, nsrc],
                                     in_=lp[:, 0:ns_],
                                     func=AF.Tanh, scale=0.5)
                _exp_piece(E_T, fr["Bt"], fr["R"], E, nsrc, n, ns_)

            if last:
                # normalize: E' = E / den, then full f32 chain + output DMA
                den = sml.tile([128, NS, NBLK], F32, tag="den")
                nc.vector.tensor_reduce(den[:, 0:ns_], E[:, 0:ns_, :, 0:n],
                                        axis=AX.X, op=ALU.add)
                rd = sml.tile([128, NS, NBLK], F32, tag="rd")
                nc.vector.reciprocal(rd[:, 0:ns_], den[:, 0:ns_])
                Ew = sml.tile([128, NS, NBLK, 5], F32, tag="Ew")
                nc.vector.scalar_tensor_tensor(
                    out=Ew[:, 0:ns_, :, 0:n], in0=E[:, 0:ns_, :, 0:n],
                    scalar=1.0, in1=_bcast(rd[:, 0:ns_], n),
                    op0=ALU.bypass, op1=ALU.mult)
                for st in sts:
                    k, slots = st["k"], st["slots"]
                    h = big.tile([128, NBLK, D], F32, tag="h_f32", name="h")
                    for blk in range(NBLK):
                        nc.vector.tensor_scalar(
                            out=h[:, blk, :], in0=slots[:, 0, blk, :],
                            scalar1=Ew[:, k, blk, 0:1], scalar2=None,
                            op0=ALU.mult)
                        for i in range(1, n):
                            nc.vector.scalar_tensor_tensor(
                                out=h[:, blk, :], in0=slots[:, i, blk, :],
                                scalar=Ew[:, k, blk, i:i + 1],
                                in1=h[:, blk, :],
                                op0=ALU.mult, op1=ALU.add)
                    nc.sync.dma_start(out_v[bass.ds(st["it"], 1)], h[:])
                return nxt

            hsum, hs = fr["hsum"], fr["hs"]
            if fr.get("late_static"):
                # newest block: tanh + exp now that the commit path is done
                nc.scalar.activation(out=E_T[:, 0:ns_, :, nsrc - 1],
                                     in_=sh["sdots_all"][:, 0:ns_, nsrc - 1,
                                                         :, l],
                                     func=AF.Tanh, scale=0.5)
                _exp_piece(E_T, fr["Bt"], fr["R"], E, nsrc - 1, nsrc, ns_)
            # deferred last links (accumulate hsum)
            if has_p or fr.get("late_static"):
                for st, h in zip(sts, hs):
                    k, slots = st["k"], st["slots"]
                    for blk in range(NBLK):
                        nc.vector.scalar_tensor_tensor(
                            out=h[:, blk, :], in0=slots[:, n - 1, blk, :],
                            scalar=E[:, k, blk, n - 1:n],
                            in1=h[:, blk, :],
                            op0=ALU.mult, op1=ALU.add,
                            accum_out=hsum[:, k, blk:blk + 1])

            # --- LN stats: var = hssq/D - mu^2 ---
            hssq = sml.tile([128, NS, NBLK], F32, tag="hssq")
            for st, h in zip(sts, hs):
                sqh = big.tile([128, NBLK, D], BF16, tag="sq", name="sqh", bufs=8)
                emit_square(HSSQ_SQ, sqh[:], h[:])
                nc.vector.tensor_reduce(hssq[:, st["k"]], sqh[:],
                                        axis=AX.X, op=ALU.add)
            m2 = sml.tile([128, NS, NBLK], F32, tag="m2")
            nc.vector.tensor_mul(m2[:, 0:ns_], hsum[:, 0:ns_], hsum[:, 0:ns_])
            t1 = sml.tile([128, NS, NBLK], F32, tag="t1")
            nc.vector.tensor_scalar(out=t1[:, 0:ns_], in0=hssq[:, 0:ns_],
                                    scalar1=1.0 / D, scalar2=EPS_LN,
                                    op0=ALU.mult, op1=ALU.add)
            xs2 = sml.tile([128, NS, NBLK], F32, tag="xs2")
            nc.vector.scalar_tensor_tensor(
                out=xs2[:, 0:ns_], in0=m2[:, 0:ns_], scalar=-1.0 / (D * D),
                in1=t1[:, 0:ns_], op0=ALU.mult, op1=ALU.add)
            s_ln = _newton_rsqrt(nc, nwp, xs2, (NS, NBLK), iters=1)
            mu = sml.tile([128, NS, NBLK], F32, tag="mu")
            nc.vector.tensor_scalar_mul(mu[:, 0:ns_], hsum[:, 0:ns_], 1.0 / D)

            # --- LN apply + MLP, phased across streams ---
            xns, xnTs, xnT_pss, Gs, vT_pss, vtTs = [], [], [], [], [], []
            for st, h in zip(sts, hs):
                k = st["k"]
                xn = big.tile([128, NBLK, D], BF16, tag="xn", name="xn")
                xns.append(xn)
                for blk in range(NBLK):
                    nc.vector.tensor_scalar(
                        out=xn[:, blk, :], in0=h[:, blk, :],
                        scalar1=mu[:, k, blk:blk + 1],
                        scalar2=s_ln[:, k, blk:blk + 1],
                        op0=ALU.subtract, op1=ALU.mult)
            for st, xn in zip(sts, xns):
                xnT_ps = pp_big.tile([128, F], BF16, tag="big_ps",
                                     name="xnT_ps")
                xnT_pss.append(xnT_ps)
                for blk in range(NBLK):
                    nc.tensor.matmul(xnT_ps[:, blk * 128:(blk + 1) * 128],
                                     xn[:, blk, :], ident16[:],
                                     is_transpose=True, start=True, stop=True,
                                     skip_group_check=True)
            for st, xnT_ps in zip(sts, xnT_pss):
                xnT = big.tile([128, F], BF16, tag="xnT", name="xnT")
                xnTs.append(xnT)
                nc.vector.tensor_copy(xnT[:], xnT_ps[:])
            h1s = []
            for st, xnT in zip(sts, xnTs):
                hh = []
                for half in range(2):
                    h1 = pp_big.tile([128, F], F32, tag="big_ps", name="h1")
                    nc.tensor.matmul(h1[:], w1p_sb[:, l, half, :], xnT[:],
                                     start=True, stop=True,
                                     skip_group_check=True)
                    hh.append(h1)
                h1s.append(hh)
            for st, hh in zip(sts, h1s):
                G = []
                for half in range(2):
                    gh = big.tile([128, F], BF16, tag=f"g{half}", name="gh")
                    nc.scalar.activation(
                        gh[:], hh[half][:], AF.Gelu,
                        bias=b1p_sb[:, 2 * l + half:2 * l + half + 1])
                    G.append(gh)
                Gs.append(G)
            for st, G in zip(sts, Gs):
                vT_ps = pp_big.tile([128, F], F32, tag="big_ps", name="vT_ps")
                vT_pss.append(vT_ps)
                for kh in range(2):
                    nc.tensor.matmul(vT_ps[:], w2p_sb[:, l, kh, :], G[kh][:],
                                     start=(kh == 0), stop=(kh == 1),
                                     skip_group_check=True)
            for st, vT_ps in zip(sts, vT_pss):
                vtT = big.tile([128, F], F32R, tag="vtT", name="vtT")
                vtTs.append(vtT)
                if VTT_SPLIT and (l % 2 == 1):
                    nc.vector.tensor_copy(vtT[:], vT_ps[:])
                else:
                    nc.scalar.copy(vtT[:], vT_ps[:])
            for st, vtT in zip(sts, vtTs):
                k = st["k"]
                # running dots of partial vs every layer's w, PSUM-accumulated
                # (stream k's banded lhsT adds zeros outside its 16 rows)
                nc.tensor.matmul(dots_psum[:], wT64[:, k, :],
                                 vtT[:], start=(j == 0 and k == 0),
                                 stop=True, skip_group_check=True)
                partial_ps = st["partial_ps"]
                for blk in range(NBLK):
                    nc.tensor.matmul(partial_ps[:, blk, :],
                                     vtT[:, blk * 128:(blk + 1) * 128],
                                     identr[:], is_transpose=True,
                                     start=(j == 0 and blk == 0),
                                     stop=(j == GROUP - 1 or l == L - 2),
                                     skip_group_check=True)
            for st in sts:
                nc.scalar.copy(st["slots"][:, g + 1, :, :],
                               st["partial_ps"][:])
            if j == GROUP - 1:
                commit_creation(sts, g + 1, sh, dots_psum)
            return nxt

        spd = es.enter_context(tc.tile_pool(name="spd", bufs=1))
        with tc.For_i(0, tiles_per_core // NS, 1,
              hint_engines=(mybir.EngineType.DVE,
                            mybir.EngineType.Activation,
                            mybir.EngineType.PE,
                            mybir.EngineType.Pool)) as it0:
            sh = {}
            sh["sdots_all"] = spd.tile([128, NS, 5, NBLK, L], F32,
                                       tag="sdots_all", name="sdots_all")
            sts = [tile_start(it0 * NS + k, k, sh) for k in range(NS)]
            emb_creation(sts, sh)
            dots_psum = pp_dots.tile([64, F], F32, tag="dots_psum",
                                     name="dots_psum")
            fr = layer_front(sts, 0, sh)
            for l in range(L):
                filler = None
                nxt = layer_back(sts, l, sh, dots_psum, fr, filler)
                if nxt is None and l + 1 < L:
                    nxt = layer_front(sts, l + 1, sh)
                fr = nxt

    nc.finalize()
    return nc


def _prep_consts(w, ln_g, ln_b, W1, b1, W2):
    bf = ml_dtypes.bfloat16
    W1p = ln_g[:, :, None] * W1                                   # diag(g) @ W1
    b1p = b1 + np.einsum("ld,ldm->lm", ln_b, W1)                  # b1 + ln_b @ W1
    w1p = np.ascontiguousarray(W1p.transpose(1, 0, 2)).reshape(D, L * 2 * 128)
    b1p_sb = b1p.reshape(L, 2, 128).transpose(2, 0, 1).reshape(128, 2 * L)
    w2p = W2.reshape(L, 2, 128, D).transpose(2, 0, 1, 3)
    w2p = np.ascontiguousarray(w2p).reshape(128, L * 2 * D)
    wt = np.ascontiguousarray(w.T)
    return {
        "w_tf": wt.astype(np.float32),
        "w_tb": wt.astype(bf),
        "w1p": w1p.astype(bf),
        "b1p": np.ascontiguousarray(b1p_sb).astype(np.float32),
        "w2p": w2p.astype(bf),
    }


def kernel(embedding, w, ln_g, ln_b, W1, b1, W2, b2, _tiles=16, _trace=False):
    if _trace:
        _install_ntff_hook()
    B, T, Dd = embedding.shape
    assert Dd == D
    n_tok = _tiles * F

    key = ("k", _tiles)
    if key not in _CACHE:
        _CACHE[key] = build(_tiles)
    nc = _CACHE[key]

    assert np.all(np.asarray(b2) == 0.0), "nonzero b2 unsupported"
    consts = _prep_consts(np.asarray(w, np.float32),
                          np.asarray(ln_g, np.float32),
                          np.asarray(ln_b, np.float32),
                          np.asarray(W1, np.float32),
                          np.asarray(b1, np.float32),
                          np.asarray(W2, np.float32))
    emb_full = np.asarray(embedding, np.float32).reshape(B * T, D)
    emb_bf = emb_full.astype(ml_dtypes.bfloat16)

    per_core = B * T // N_CORES
    in_maps = []
    for c in range(N_CORES):
        shard = emb_bf[c * per_core:(c + 1) * per_core][:n_tok]
        in_maps.append({"embh": np.ascontiguousarray(shard), **consts})

    res = run_bass_kernel_spmd(nc, in_maps, core_ids=list(range(N_CORES)),
                               trace=_trace)
    outs = [res.results[c]["out"] for c in range(N_CORES)]
    full = np.stack(outs).reshape(N_CORES, n_tok, D)
    kernel.last_exec_ns = getattr(res, "exec_time_ns", None)
    kernel.last_mean_ns = getattr(res, "mean_exec_time_ns", None)
    if n_tok == per_core:
        return full.reshape(B, T, D)
    return full  # debug partial run


# revision 48
# speedup vs baseline: 1.1964x; 1.1964x over previous
"""Trainium2 Bass kernel for nn_BlockAttnRes (block-softmax residual net).

Shapes: embedding [8, 8192, 128] f32, L=16 layers, BLOCK_SIZE=4.
Sharding: batch dim B=8 across 8 cores (1 batch row / core = 8192 tokens).

Per-core: tokens-on-partitions bf16 state resident in SBUF.
6 state slots: slot0 = emb, slot 1+g = partial of group g (becomes block g+1
at commit). Loop: For_i over token tiles (F=512 tokens = 4 blocks of 128),
python-unrolled 16 layers inside. Key techniques:
  - softmax denominator never materialized: LayerNorm is scale-invariant, so
    the unnormalized h~ = sum_i E_i V_i feeds LN directly (den only enters an
    eps correction and the final layer's output scaling).
  - exp via tanh identity e^t=(1+T)/(1-T) (gelu ACT table set only)
  - running partial's logits vs EVERY layer's w via one PE matmul per layer
    on the transposed MLP output, accumulated across layers directly in PSUM
    (start=False accumulation; all 4 streams stacked in one PSUM bank at
    partition offsets 0/32/64/96). At block-commit the accumulated dots ARE
    the new block's static logits, so creation needs no transposes/matmuls.
  - rsqrt via int bit-trick seed + Newton iterations (DVE only)
  - weighted sums: per-block fused stt chains, last link's accum_out gives
    hsum free; one block's chain runs on the idle Pool engine.
  - LayerNorm affine folded into W1' = diag(g)@W1, b1' = b1 + ln_b@W1 (host)
  - MLP: PE transposes to col layout, bf16 matmuls, ACT gelu fused bias
  - partial accumulated in PSUM by f32r transpose-matmuls (start=False accum)
"""
import contextlib
import ctypes
import sys
import types
from contextlib import ExitStack

sys.path.insert(0, "/opt/trn_rl_repo")


def _install_ntff_hook():
    """Provide antenv.axon_hooks (missing in the trimmed repo) so
    run_bass_kernel_spmd(trace=True) can collect NTFF profiles."""
    if "antenv.axon_hooks" in sys.modules:
        return
    try:
        lib = ctypes.CDLL("/opt/axon/libaxon_pjrt.so")
    except OSError:
        return
    if not hasattr(lib, "axon_start_nrt_profile"):
        hook = None
    else:
        lib.axon_start_nrt_profile.argtypes = [
            ctypes.POINTER(ctypes.c_int64), ctypes.c_size_t]
        lib.axon_start_nrt_profile.restype = ctypes.c_int64
        lib.axon_stop_nrt_profile.argtypes = [ctypes.c_char_p]
        lib.axon_stop_nrt_profile.restype = ctypes.c_int64

        @contextlib.contextmanager
        def hook(output_dir, device_ids):
            import jax
            jax.devices()
            if device_ids:
                ids = (ctypes.c_int64 * len(device_ids))(*device_ids)
                rc = lib.axon_start_nrt_profile(ids, len(device_ids))
            else:
                rc = lib.axon_start_nrt_profile(None, 0)
            if rc != 0:
                raise RuntimeError(f"axon_start_nrt_profile rc={rc}")
            try:
                yield
            finally:
                n = lib.axon_stop_nrt_profile(str(output_dir).encode())
                print(f"profile: {n} file(s) -> {output_dir}", file=sys.stderr)

    mod = types.ModuleType("antenv.axon_hooks")
    mod.get_axon_ntff_profile_hook = lambda: hook
    mod.set_axon_ntff_profile_hook = lambda h: None
    sys.modules["antenv.axon_hooks"] = mod

import numpy as np
import ml_dtypes

import concourse.bacc as bacc
import concourse.bass as bass
import concourse.mybir as mybir
from concourse.bass_utils import run_bass_kernel_spmd
from concourse.tile import TileContext
from concourse.masks import make_identity

F32 = mybir.dt.float32
BF16 = mybir.dt.bfloat16
F32R = mybir.dt.float32r
I32 = mybir.dt.int32
ALU = mybir.AluOpType
AF = mybir.ActivationFunctionType
AX = mybir.AxisListType

L = 16
GROUP = 4
D = 128
NBLK = 4                 # 128-token blocks per tile
F = NBLK * 128           # tokens per tile
EPS_RMS = 1e-8
EPS_LN = 1e-5
MAGIC = 0x5F3759DF
N_CORES = 8

# --- tunable engine placement ------------------------------------------
POOL_WSUM_BLKS = 1       # how many of the 4 blocks' wsum chains go to Pool
HSSQ_SQ = "act"          # square for LN variance: "act" | "dve" | "pool"
SSQP_SQ = "pool"          # square for partial rms
VTT_SPLIT = True         # alternate vtT copy between ACT and DVE

_CACHE = {}


def _bcast(ap, n):
    """Append a stride-0 inner free dim of size n to an AP."""
    return bass.AP(tensor=ap.tensor, offset=ap.offset,
                   ap=list(ap.ap) + [[0, n]])


def _newton_rsqrt(nc, pool, x, shape, iters=2):
    """y = rsqrt(x) for x [128, *shape] f32 tile (positive). DVE-only:
    int bit-trick seed + Newton iterations. (ACT Sqrt would force a
    1.3us activation-table reload against the resident gelu table.)"""
    y = pool.tile([128] + list(shape), F32, tag="nw_y", name="nw_y")
    xi = x.bitcast(I32)
    yi = y.bitcast(I32)
    nc.vector.tensor_scalar(out=yi[:], in0=xi[:], scalar1=1, scalar2=0,
                            op0=ALU.logical_shift_right,
                            op1=ALU.logical_shift_right)
    nc.vector.tensor_scalar(out=yi[:], in0=yi[:], scalar1=-1, scalar2=MAGIC,
                            op0=ALU.mult, op1=ALU.add)
    t = pool.tile([128] + list(shape), F32, tag="nw_t", name="nw_t")
    for _ in range(iters):
        nc.vector.tensor_mul(t[:], y[:], y[:])
        nc.vector.scalar_tensor_tensor(out=t[:], in0=t[:], scalar=-0.5,
                                       in1=x[:], op0=ALU.mult, op1=ALU.mult)
        nc.vector.scalar_tensor_tensor(out=y[:], in0=t[:], scalar=1.5,
                                       in1=y[:], op0=ALU.add, op1=ALU.mult)
    return y


def build(tiles_per_core=16):
    nc = bacc.Bacc("TRN2", target_bir_lowering=False)
    n_tok = tiles_per_core * F

    embh = nc.dram_tensor("embh", [n_tok, D], BF16, kind="ExternalInput")
    w_tf = nc.dram_tensor("w_tf", [D, L], F32, kind="ExternalInput")
    w_tb = nc.dram_tensor("w_tb", [D, L], BF16, kind="ExternalInput")
    w1p = nc.dram_tensor("w1p", [D, L * 2 * 128], BF16, kind="ExternalInput")
    b1p = nc.dram_tensor("b1p", [128, 2 * L], F32, kind="ExternalInput")
    w2p = nc.dram_tensor("w2p", [128, L * 2 * D], BF16, kind="ExternalInput")
    out = nc.dram_tensor("out", [n_tok, D], F32, kind="ExternalOutput")

    emb_v = embh.rearrange("(T b p) d -> T p b d", b=NBLK, p=128)
    out_v = out.rearrange("(T b p) d -> T p b d", b=NBLK, p=128)

    with TileContext(nc) as tc, ExitStack() as es:
        cst = es.enter_context(tc.tile_pool(name="cst", bufs=1))
        ident16 = cst.tile([128, 128], BF16)
        make_identity(nc, ident16[:])
        identf = cst.tile([128, 128], F32)
        make_identity(nc, identf[:])
        identr = cst.tile([128, 128], F32R)
        nc.vector.tensor_copy(identr[:], identf[:])

        wT_f = cst.tile([128, L], F32)
        nc.sync.dma_start(wT_f[:], w_tf[:])
        wT_b = cst.tile([128, L], BF16)
        nc.sync.dma_start(wT_b[:], w_tb[:])
        # per-stream banded [128, 64] lhsT: w.T in cols 16k..16k+15, 0 else
        wT64_f = cst.tile([128, 4, 64], F32)
        nc.vector.memset(wT64_f[:], 0.0)
        for k4 in range(4):
            nc.vector.tensor_copy(wT64_f[:, k4, 16 * k4:16 * k4 + L], wT_f[:])
        wT64 = cst.tile([128, 4, 64], F32R)
        nc.vector.tensor_copy(wT64[:], wT64_f[:])

        w1p_sb = cst.tile([128, L, 2, 128], BF16)
        nc.sync.dma_start(w1p_sb[:], w1p[:].rearrange(
            "d (l h m) -> d l h m", l=L, h=2))
        b1p_sb = cst.tile([128, 2 * L], F32)
        nc.sync.dma_start(b1p_sb[:], b1p[:])
        w2p_sb = cst.tile([128, L, 2, D], BF16)
        nc.sync.dma_start(w2p_sb[:], w2p[:].rearrange(
            "m (l k d) -> m l k d", l=L, k=2))

        sp = es.enter_context(tc.tile_pool(name="state", bufs=4))
        big = es.enter_context(tc.tile_pool(name="big", bufs=5))
        sml = es.enter_context(tc.tile_pool(name="sml", bufs=12))
        nwp = es.enter_context(tc.tile_pool(name="nw", bufs=12))
        pp_big = es.enter_context(tc.tile_pool(name="pp_big", bufs=3, space="PSUM"))
        pp_par = es.enter_context(tc.tile_pool(name="pp_par", bufs=4, space="PSUM"))
        pp_dots = es.enter_context(tc.tile_pool(name="pp_dots", bufs=1, space="PSUM"))

        NS = 4 if tiles_per_core % 4 == 0 else (
            2 if tiles_per_core % 2 == 0 else 1)

        def sq_engine(which):
            return {"act": None, "dve": nc.vector, "pool": nc.gpsimd}[which]

        def emit_square(which, out_ap, in_ap):
            if which == "act":
                nc.scalar.activation(out_ap, in_ap, AF.Square)
            else:
                eng = nc.vector if which == "dve" else nc.gpsimd
                eng.tensor_tensor(out_ap, in_ap, in_ap, op=ALU.mult)

        def tile_start(it, k, sh):
            st = {"it": it, "k": k, "sh": sh}
            st["slots"] = sp.tile([128, 6, NBLK, D], BF16, tag="slots",
                                  name="slots")
            nc.sync.dma_start(st["slots"][:, 0, :, :], emb_v[bass.ds(it, 1)])
            st["partial_ps"] = pp_par.tile([128, NBLK, D], F32R, tag="par",
                                           name="par")
            return st

        def stats_finish(sts, s_idx, sh, stats_cr, ms_cr):
            """Batched: rms from ms + scaled dots -> sdots_all[s_idx]."""
            ns_ = len(sts)
            xs = sml.tile([128, NS, NBLK], F32, tag="xs_cr")
            nc.vector.tensor_scalar(out=xs[:, 0:ns_], in0=ms_cr[:, 0:ns_],
                                    scalar1=1.0 / D, scalar2=EPS_RMS,
                                    op0=ALU.mult, op1=ALU.add)
            rms = _newton_rsqrt(nc, nwp, xs, (NS, NBLK), iters=1)
            nc.vector.scalar_tensor_tensor(
                out=sh["sdots_all"][:, 0:ns_, s_idx, :, :],
                in0=stats_cr[:, 0:ns_, :, 0:L],
                scalar=1.0, in1=_bcast(rms[:, 0:ns_], L),
                op0=ALU.bypass, op1=ALU.mult)

        def emb_creation(sts, sh):
            """Stats for slot 0 (embedding): transpose + dots matmul +
            token-layout mean-square."""
            stats_cr = sml.tile([128, NS, NBLK, L], F32, tag="stats_cr")
            ms_cr = sml.tile([128, NS, NBLK], F32, tag="ms_cr")
            for st in sts:
                k = st["k"]
                srcT_ps = pp_big.tile([128, F], BF16, tag="big_ps",
                                      name="srcT_ps")
                for blk in range(NBLK):
                    nc.tensor.matmul(srcT_ps[:, blk * 128:(blk + 1) * 128],
                                     st["slots"][:, 0, blk, :], ident16[:],
                                     is_transpose=True, start=True, stop=True,
                                     skip_group_check=True)
                srcT = big.tile([128, F], BF16, tag="srcT", name="srcT")
                nc.scalar.copy(srcT[:], srcT_ps[:])
                dots_ps = pp_big.tile([L, F], F32, tag="big_ps",
                                      name="dots_ps")
                nc.tensor.matmul(dots_ps[:], wT_b[:], srcT[:],
                                 start=True, stop=True, skip_group_check=True)
                dots_sb = big.tile([L, F], F32, tag="dots_sb",
                                   name="dots_sb")
                nc.scalar.copy(dots_sb[:], dots_ps[:])
                statT_ps = pp_big.tile([128, NBLK, L], F32, tag="big_ps",
                                       name="statT_ps")
                for c in range(NBLK):
                    nc.tensor.matmul(statT_ps[:, c, :],
                                     dots_sb[:, c * 128:(c + 1) * 128],
                                     identf[0:L, 0:L],
                                     is_transpose=True, start=True, stop=True,
                                     skip_group_check=True)
                nc.vector.tensor_copy(stats_cr[:, k], statT_ps[:])
                sqe = big.tile([128, NBLK, D], BF16, tag="sq", name="sqe", bufs=8)
                emit_square("act", sqe[:], st["slots"][:, 0, :, :])
                nc.vector.tensor_reduce(ms_cr[:, k], sqe[:],
                                        axis=AX.X, op=ALU.add)
            stats_finish(sts, 0, sh, stats_cr, ms_cr)

        def commit_creation(sts, s_idx, sh, dots_psum):
            """Block commit: the accumulated dots ARE the new block's static
            logits; only rms (token-layout) + transpose needed."""
            ns_ = len(sts)
            stats_cr = sml.tile([128, NS, NBLK, L], F32, tag="stats_cr")
            ms_cr = sml.tile([128, NS, NBLK], F32, tag="ms_cr")
            drow = big.tile([64, F], F32, tag="drow", name="drow")
            nc.scalar.copy(drow[:], dots_psum[:])
            statT_ps = pp_big.tile([128, NBLK, 64], F32, tag="big_ps",
                                   name="statT_ps")
            for c in range(NBLK):
                nc.tensor.matmul(statT_ps[:, c, :],
                                 drow[:, c * 128:(c + 1) * 128],
                                 identf[0:64, 0:64],
                                 is_transpose=True, start=True, stop=True,
                                 skip_group_check=True)
            sp_ap = statT_ps[:]
            nc.vector.tensor_copy(
                stats_cr[:, 0:ns_],
                bass.AP(tensor=sp_ap.tensor, offset=sp_ap.offset,
                        ap=[sp_ap.ap[0], [16, ns_], [64, NBLK], [1, L]]))
            for st in sts:
                sqc = big.tile([128, NBLK, D], BF16, tag="sq", name="sqc", bufs=8)
                nc.scalar.activation(sqc[:], st["slots"][:, s_idx, :, :],
                                     AF.Square)
                nc.vector.tensor_reduce(ms_cr[:, st["k"]], sqc[:],
                                        axis=AX.X, op=ALU.add)
            stats_finish(sts, s_idx, sh, stats_cr, ms_cr)

        def _exp_piece(E_T, Bt, R, E, lo, hi, ns_):
            """E[.., lo:hi] = exp from T = tanh(logit/2)."""
            nc.vector.tensor_scalar(out=Bt[:, 0:ns_, :, lo:hi],
                                    in0=E_T[:, 0:ns_, :, lo:hi],
                                    scalar1=-1.0, scalar2=-1.0,
                                    op0=ALU.mult, op1=ALU.subtract)
            nc.vector.reciprocal(R[:, 0:ns_, :, lo:hi],
                                 Bt[:, 0:ns_, :, lo:hi])
            nc.vector.tensor_scalar(out=E[:, 0:ns_, :, lo:hi],
                                    in0=R[:, 0:ns_, :, lo:hi],
                                    scalar1=2.0, scalar2=-1.0,
                                    op0=ALU.mult, op1=ALU.add)

        def layer_front(sts, l, sh):
            """Static part of a layer: softmax statics + static chain links.
            Depends only on committed slots/sdots, so it can be emitted ahead
            to keep engines fed while the previous layer's cross-engine
            round-trips resolve."""
            ns_ = len(sts)
            g, j = l // GROUP, l % GROUP
            nsrc = g + 1
            has_p = j > 0
            n = nsrc + (1 if has_p else 0)
            last = l == L - 1
            sdots_all = sh["sdots_all"]

            E_T = sml.tile([128, NS, NBLK, 5], F32, tag="E_T")
            e_ap = E_T[:]
            statics_out = bass.AP(
                tensor=e_ap.tensor, offset=e_ap.offset,
                ap=[e_ap.ap[0], [NBLK * 5, ns_], [1, nsrc], [5, NBLK]])
            nc.scalar.activation(out=statics_out,
                                 in_=sdots_all[:, 0:ns_, 0:nsrc, :, l],
                                 func=AF.Tanh, scale=0.5)
            Bt = sml.tile([128, NS, NBLK, 5], F32, tag="B")
            R = sml.tile([128, NS, NBLK, 5], F32, tag="R")
            E = sml.tile([128, NS, NBLK, 5], F32, tag="E")
            _exp_piece(E_T, Bt, R, E, 0, nsrc, ns_)

            fr = {"E_T": E_T, "Bt": Bt, "R": R, "E": E, "n": n,
                  "nsrc": nsrc, "has_p": has_p, "last": last}
            if last:
                return fr

            hsum = sml.tile([128, NS, NBLK], F32, tag="hsum")
            pool_blks = (range(NBLK - POOL_WSUM_BLKS, NBLK)
                         if n >= 3 else ())
            hs = []
            for st in sts:
                hs.append(big.tile([128, NBLK, D], BF16, tag="h",
                                   name="h", bufs=8))
            fr["hsum"], fr["hs"], fr["pool_blks"] = hsum, hs, pool_blks
            # phase 1: first links
            for st, h in zip(sts, hs):
                k, slots = st["k"], st["slots"]
                for blk in range(NBLK):
                    if n == 1:
                        nc.vector.tensor_scalar(
                            out=h[:, blk, :], in0=slots[:, 0, blk, :],
                            scalar1=E[:, k, blk, 0:1], scalar2=0.0,
                            op0=ALU.mult, op1=ALU.add,
                            accum_out=hsum[:, k, blk:blk + 1])
                    elif blk in pool_blks:
                        nc.gpsimd.tensor_tensor(
                            h[:, blk, :], slots[:, 0, blk, :],
                            _bcast(E[:, k, blk, 0], 128), op=ALU.mult)
                    else:
                        nc.vector.tensor_scalar(
                            out=h[:, blk, :], in0=slots[:, 0, blk, :],
                            scalar1=E[:, k, blk, 0:1], scalar2=None,
                            op0=ALU.mult)
            # phase 2: static mid links (i in 1..n-2 are always static)
            for st, h in zip(sts, hs):
                k, slots = st["k"], st["slots"]
                for blk in range(NBLK):
                    if blk in pool_blks:
                        for i in range(1, n - 1):
                            wt = big.tile([128, 128], BF16, tag="wtmp",
                                          name="wtmp")
                            nc.gpsimd.tensor_tensor(
                                wt[:], slots[:, i, blk, :],
                                _bcast(E[:, k, blk, i], 128), op=ALU.mult)
                            nc.gpsimd.tensor_tensor(
                                h[:, blk, :], h[:, blk, :], wt[:],
                                op=ALU.add)
                    else:
                        for i in range(1, n - 1):
                            nc.vector.scalar_tensor_tensor(
                                out=h[:, blk, :], in0=slots[:, i, blk, :],
                                scalar=E[:, k, blk, i:i + 1],
                                in1=h[:, blk, :],
                                op0=ALU.mult, op1=ALU.add)
            # static last link (j == 0 layers): finish + hsum here
            if n > 1 and not has_p:
                for st, h in zip(sts, hs):
                    k, slots = st["k"], st["slots"]
                    for blk in range(NBLK):
                        nc.vector.scalar_tensor_tensor(
                            out=h[:, blk, :], in0=slots[:, n - 1, blk, :],
                            scalar=E[:, k, blk, n - 1:n],
                            in1=h[:, blk, :],
                            op0=ALU.mult, op1=ALU.add,
                            accum_out=hsum[:, k, blk:blk + 1])
            return fr

        def layer_back(sts, l, sh, dots_psum, fr, filler=None):
            ns_ = len(sts)
            g, j = l // GROUP, l % GROUP
            nsrc = g + 1
            has_p = j > 0
            n = nsrc + (1 if has_p else 0)
            last = l == L - 1
            E_T, E = fr["E_T"], fr["E"]

            # --- partial-source logit from PSUM-accumulated dots ---
            # producers on ACT/PE/Pool first, then the next layer's static
            # work as filler, then the DVE consumers (so the in-order DVE
            # queue has useful work while squares/copies land).
            if has_p:
                dotp = sml.tile([128, NS, NBLK], F32, tag="dotp")
                ssqp = sml.tile([128, NS, NBLK], F32, tag="ssqp")
                drow = big.tile([64, F], F32, tag="drow", name="drow")
                nc.scalar.copy(drow[:], dots_psum[:])
                statp_ps = pp_big.tile([128, NBLK, 64], F32, tag="big_ps",
                                       name="statp_ps")
                for c in range(NBLK):
                    nc.tensor.matmul(statp_ps[:, c, :],
                                     drow[:, c * 128:(c + 1) * 128],
                                     identf[0:64, 0:64],
                                     is_transpose=True, start=True,
                                     stop=True, skip_group_check=True)
                sqps = []
                for st in sts:
                    sqp = big.tile([128, NBLK, D], BF16, tag="sq", name="sqp", bufs=8)
                    emit_square(SSQP_SQ, sqp[:], st["slots"][:, nsrc, :, :])
                    sqps.append(sqp)
            nxt = filler() if filler is not None else None
            if has_p:
                sp_ap = statp_ps[:]
                nc.vector.tensor_copy(
                    dotp[:, 0:ns_],
                    bass.AP(tensor=sp_ap.tensor, offset=sp_ap.offset + l,
                            ap=[sp_ap.ap[0], [16, ns_], [64, NBLK]]))
                for st, sqp in zip(sts, sqps):
                    nc.vector.tensor_reduce(ssqp[:, st["k"]], sqp[:],
                                            axis=AX.X, op=ALU.add)
                xp = sml.tile([128, NS, NBLK], F32, tag="xp")
                nc.vector.tensor_scalar(out=xp[:, 0:ns_], in0=ssqp[:, 0:ns_],
                                        scalar1=1.0 / D, scalar2=EPS_RMS,
                                        op0=ALU.mult, op1=ALU.add)
                rmsp = _newton_rsqrt(nc, nwp, xp, (NS, NBLK), iters=1)
                lp = sml.tile([128, NS, NBLK], F32, tag="lp")
                nc.vector.tensor_mul(lp[:, 0:ns_], dotp[:, 0:ns_],
                                     rmsp[:, 0:ns_])
                nc.scalar.activation(out=E_T[:, 0:ns_, :, nsrc],
                                     in_=lp[:, 0:ns_],
                                     func=AF.Tanh, scale=0.5)
                _exp_piece(E_T, fr["Bt"], fr["R"], E, nsrc, n, ns_)

            if last:
                # normalize: E' = E / den, then full f32 chain + output DMA
                den = sml.tile([128, NS, NBLK], F32, tag="den")
                nc.vector.tensor_reduce(den[:, 0:ns_], E[:, 0:ns_, :, 0:n],
                                        axis=AX.X, op=ALU.add)
                rd = sml.tile([128, NS, NBLK], F32, tag="rd")
                nc.vector.reciprocal(rd[:, 0:ns_], den[:, 0:ns_])
                Ew = sml.tile([128, NS, NBLK, 5], F32, tag="Ew")
                nc.vector.scalar_tensor_tensor(
                    out=Ew[:, 0:ns_, :, 0:n], in0=E[:, 0:ns_, :, 0:n],
                    scalar=1.0, in1=_bcast(rd[:, 0:ns_], n),
                    op0=ALU.bypass, op1=ALU.mult)
                for st in sts:
                    k, slots = st["k"], st["slots"]
                    h = big.tile([128, NBLK, D], F32, tag="h_f32", name="h")
                    for blk in range(NBLK):
                        nc.vector.tensor_scalar(
                            out=h[:, blk, :], in0=slots[:, 0, blk, :],
                            scalar1=Ew[:, k, blk, 0:1], scalar2=None,
                            op0=ALU.mult)
                        for i in range(1, n):
                            nc.vector.scalar_tensor_tensor(
                                out=h[:, blk, :], in0=slots[:, i, blk, :],
                                scalar=Ew[:, k, blk, i:i + 1],
                                in1=h[:, blk, :],
                                op0=ALU.mult, op1=ALU.add)
                    nc.sync.dma_start(out_v[bass.ds(st["it"], 1)], h[:])
                return nxt

            hsum, hs = fr["hsum"], fr["hs"]
            # partial last links (accumulate hsum)
            if has_p:
                for st, h in zip(sts, hs):
                    k, slots = st["k"], st["slots"]
                    for blk in range(NBLK):
                        nc.vector.scalar_tensor_tensor(
                            out=h[:, blk, :], in0=slots[:, n - 1, blk, :],
                            scalar=E[:, k, blk, n - 1:n],
                            in1=h[:, blk, :],
                            op0=ALU.mult, op1=ALU.add,
                            accum_out=hsum[:, k, blk:blk + 1])

            # --- LN stats: var = hssq/D - mu^2 ---
            hssq = sml.tile([128, NS, NBLK], F32, tag="hssq")
            for st, h in zip(sts, hs):
                sqh = big.tile([128, NBLK, D], BF16, tag="sq", name="sqh", bufs=8)
                emit_square(HSSQ_SQ, sqh[:], h[:])
                nc.vector.tensor_reduce(hssq[:, st["k"]], sqh[:],
                                        axis=AX.X, op=ALU.add)
            m2 = sml.tile([128, NS, NBLK], F32, tag="m2")
            nc.vector.tensor_mul(m2[:, 0:ns_], hsum[:, 0:ns_], hsum[:, 0:ns_])
            t1 = sml.tile([128, NS, NBLK], F32, tag="t1")
            nc.vector.tensor_scalar(out=t1[:, 0:ns_], in0=hssq[:, 0:ns_],
                                    scalar1=1.0 / D, scalar2=EPS_LN,
                                    op0=ALU.mult, op1=ALU.add)
            xs2 = sml.tile([128, NS, NBLK], F32, tag="xs2")
            nc.vector.scalar_tensor_tensor(
                out=xs2[:, 0:ns_], in0=m2[:, 0:ns_], scalar=-1.0 / (D * D),
                in1=t1[:, 0:ns_], op0=ALU.mult, op1=ALU.add)
            s_ln = _newton_rsqrt(nc, nwp, xs2, (NS, NBLK), iters=1)
            mu = sml.tile([128, NS, NBLK], F32, tag="mu")
            nc.vector.tensor_scalar_mul(mu[:, 0:ns_], hsum[:, 0:ns_], 1.0 / D)

            # --- LN apply + MLP, phased across streams ---
            xns, xnTs, xnT_pss, Gs, vT_pss, vtTs = [], [], [], [], [], []
            for st, h in zip(sts, hs):
                k = st["k"]
                xn = big.tile([128, NBLK, D], BF16, tag="xn", name="xn")
                xns.append(xn)
                for blk in range(NBLK):
                    nc.vector.tensor_scalar(
                        out=xn[:, blk, :], in0=h[:, blk, :],
                        scalar1=mu[:, k, blk:blk + 1],
                        scalar2=s_ln[:, k, blk:blk + 1],
                        op0=ALU.subtract, op1=ALU.mult)
            for st, xn in zip(sts, xns):
                xnT_ps = pp_big.tile([128, F], BF16, tag="big_ps",
                                     name="xnT_ps")
                xnT_pss.append(xnT_ps)
                for blk in range(NBLK):
                    nc.tensor.matmul(xnT_ps[:, blk * 128:(blk + 1) * 128],
                                     xn[:, blk, :], ident16[:],
                                     is_transpose=True, start=True, stop=True,
                                     skip_group_check=True)
            for st, xnT_ps in zip(sts, xnT_pss):
                xnT = big.tile([128, F], BF16, tag="xnT", name="xnT")
                xnTs.append(xnT)
                nc.vector.tensor_copy(xnT[:], xnT_ps[:])
            h1s = []
            for st, xnT in zip(sts, xnTs):
                hh = []
                for half in range(2):
                    h1 = pp_big.tile([128, F], F32, tag="big_ps", name="h1")
                    nc.tensor.matmul(h1[:], w1p_sb[:, l, half, :], xnT[:],
                                     start=True, stop=True,
                                     skip_group_check=True)
                    hh.append(h1)
                h1s.append(hh)
            for st, hh in zip(sts, h1s):
                G = []
                for half in range(2):
                    gh = big.tile([128, F], BF16, tag=f"g{half}", name="gh")
                    nc.scalar.activation(
                        gh[:], hh[half][:], AF.Gelu,
                        bias=b1p_sb[:, 2 * l + half:2 * l + half + 1])
                    G.append(gh)
                Gs.append(G)
            for st, G in zip(sts, Gs):
                vT_ps = pp_big.tile([128, F], F32, tag="big_ps", name="vT_ps")
                vT_pss.append(vT_ps)
                for kh in range(2):
                    nc.tensor.matmul(vT_ps[:], w2p_sb[:, l, kh, :], G[kh][:],
                                     start=(kh == 0), stop=(kh == 1),
                                     skip_group_check=True)
            for st, vT_ps in zip(sts, vT_pss):
                vtT = big.tile([128, F], F32R, tag="vtT", name="vtT")
                vtTs.append(vtT)
                if VTT_SPLIT and (l % 2 == 1):
                    nc.vector.tensor_copy(vtT[:], vT_ps[:])
                else:
                    nc.scalar.copy(vtT[:], vT_ps[:])
            for st, vtT in zip(sts, vtTs):
                k = st["k"]
                # running dots of partial vs every layer's w, PSUM-accumulated
                # (stream k's banded lhsT adds zeros outside its 16 rows)
                nc.tensor.matmul(dots_psum[:], wT64[:, k, :],
                                 vtT[:], start=(j == 0 and k == 0),
                                 stop=True, skip_group_check=True)
                partial_ps = st["partial_ps"]
                for blk in range(NBLK):
                    nc.tensor.matmul(partial_ps[:, blk, :],
                                     vtT[:, blk * 128:(blk + 1) * 128],
                                     identr[:], is_transpose=True,
                                     start=(j == 0 and blk == 0),
                                     stop=(j == GROUP - 1 or l == L - 2),
                                     skip_group_check=True)
            for st in sts:
                nc.scalar.copy(st["slots"][:, g + 1, :, :],
                               st["partial_ps"][:])
            if j == GROUP - 1:
                commit_creation(sts, g + 1, sh, dots_psum)
            return nxt

        spd = es.enter_context(tc.tile_pool(name="spd", bufs=1))
        with tc.For_i(0, tiles_per_core // NS, 1,
              hint_engines=(mybir.EngineType.DVE,
                            mybir.EngineType.Activation,
                            mybir.EngineType.PE,
                            mybir.EngineType.Pool)) as it0:
            sh = {}
            sh["sdots_all"] = spd.tile([128, NS, 5, NBLK, L], F32,
                                       tag="sdots_all", name="sdots_all")
            sts = [tile_start(it0 * NS + k, k, sh) for k in range(NS)]
            emb_creation(sts, sh)
            dots_psum = pp_dots.tile([64, F], F32, tag="dots_psum",
                                     name="dots_psum")
            fr = layer_front(sts, 0, sh)
            for l in range(L):
                filler = None
                nxt = layer_back(sts, l, sh, dots_psum, fr, filler)
                if nxt is None and l + 1 < L:
                    nxt = layer_front(sts, l + 1, sh)
                fr = nxt

    nc.finalize()
    return nc


def _prep_consts(w, ln_g, ln_b, W1, b1, W2):
    bf = ml_dtypes.bfloat16
    W1p = ln_g[:, :, None] * W1                                   # diag(g) @ W1
    b1p = b1 + np.einsum("ld,ldm->lm", ln_b, W1)                  # b1 + ln_b @ W1
    w1p = np.ascontiguousarray(W1p.transpose(1, 0, 2)).reshape(D, L * 2 * 128)
    b1p_sb = b1p.reshape(L, 2, 128).transpose(2, 0, 1).reshape(128, 2 * L)
    w2p = W2.reshape(L, 2, 128, D).transpose(2, 0, 1, 3)
    w2p = np.ascontiguousarray(w2p).reshape(128, L * 2 * D)
    wt = np.ascontiguousarray(w.T)
    return {
        "w_tf": wt.astype(np.float32),
        "w_tb": wt.astype(bf),
        "w1p": w1p.astype(bf),
        "b1p": np.ascontiguousarray(b1p_sb).astype(np.float32),
        "w2p": w2p.astype(bf),
    }


def kernel(embedding, w, ln_g, ln_b, W1, b1, W2, b2, _tiles=16, _trace=False):
    if _trace:
        _install_ntff_hook()
    B, T, Dd = embedding.shape
    assert Dd == D
    n_tok = _tiles * F

    key = ("k", _tiles)
    if key not in _CACHE:
        _CACHE[key] = build(_tiles)
    nc = _CACHE[key]

    assert np.all(np.asarray(b2) == 0.0), "nonzero b2 unsupported"
    consts = _prep_consts(np.asarray(w, np.float32),
                          np.asarray(ln_g, np.float32),
                          np.asarray(ln_b, np.float32),
                          np.asarray(W1, np.float32),
                          np.asarray(b1, np.float32),
                          np.asarray(W2, np.float32))
    emb_full = np.asarray(embedding, np.float32).reshape(B * T, D)
    emb_bf = emb_full.astype(ml_dtypes.bfloat16)

    per_core = B * T // N_CORES
    in_maps = []
    for c in range(N_CORES):
        shard = emb_bf[c * per_core:(c + 1) * per_core][:n_tok]
        in_maps.append({"embh": np.ascontiguousarray(shard), **consts})

    res = run_bass_kernel_spmd(nc, in_maps, core_ids=list(range(N_CORES)),
                               trace=_trace)
    outs = [res.results[c]["out"] for c in range(N_CORES)]
    full = np.stack(outs).reshape(N_CORES, n_tok, D)
    kernel.last_exec_ns = getattr(res, "exec_time_ns", None)
    kernel.last_mean_ns = getattr(res, "mean_exec_time_ns", None)
    if n_tok == per_core:
        return full.reshape(B, T, D)
    return full  # debug partial run
